# revision 1
# baseline (speedup 1.0000x reference)
"""GPT2 attention block (B=4, S=2048, D=1280, H=20) on 8 Trainium2 cores.

Sharding: core i handles batch b = i // 2 and head-group g = i % 2
(10 of the 20 heads). Data parallel on B, tensor parallel on heads:
c_attn column-split, c_proj row-split; each core returns a partial
projection output which the host reduces (plus host-folded biases).

Per-core kernel (all matmul inputs bf16, fp32 PSUM accumulation):
  - x^T streamed per 512-query macro; qkv computed as q^T,k^T (e on
    partitions, via lhsT=w) and v natural (via lhsT=x^T).
  - causal flash-style attention per head pair: scores^T = k^T.T@q^T
    (two heads row-packed on the PE), exp on ACT (scale=1/8 folded in),
    probs@v via lhsT=[v|1] so the ones column yields softmax sums for
    free; normalization by 1/sums broadcast via a DRAM bounce.
  - projection y^T = w_proj.T @ out^T per macro, accumulated on host.

The host transposes/reassembles and adds b_proj + b_attn_v @ w_proj
(exactly equivalent since softmax rows sum to 1).
"""

import numpy as np
import ml_dtypes

import concourse.bass as bass
import concourse.mybir as mybir
import concourse.tile as tile
from concourse import bass_utils

B, S, D, H, HD = 4, 2048, 1280, 20, 64
N_CORES = 8
HL = H // 2            # heads per core
EL = HL * HD           # 640 local columns for each of q/k/v
DC = D // 128          # 10 contraction chunks over D
WC = EL // 128         # 5 chunks over local e (q or k)
MACRO = 512
NM = S // MACRO        # 4 macros
CPM = MACRO // 128     # 4 key-chunks per macro
PAIRS = HL // 2        # 5 head pairs

bf16 = mybir.dt.bfloat16
f32 = mybir.dt.float32
EXP = mybir.ActivationFunctionType.Exp

_CACHE: dict = {}


def _fix_sync_caps(nc):
    """walrus in this container accepts at most 1 sem wait / 1 sem update
    per instruction; Tile emits more (notably the end-of-context drain).
    Hoist excess waits onto NOPs inserted before the offender."""
    for f in nc.m.functions:
        for bb in f.blocks:
            insts = bb.instructions
            if not any(
                i.sync_info is not None and len(i.sync_info.on_wait) > 1
                for i in insts
            ):
                continue
            out = []
            for inst in insts:
                si = inst.sync_info
                if si is not None and len(si.on_wait) > 1:
                    waits = list(si.on_wait)
                    for w in waits[:-1]:
                        out.append(
                            mybir.InstNoOp(
                                name=f"I-{nc.next_id()}",
                                opcode="NoOp",
                                engine=inst.engine,
                                sync_info=mybir.SyncInfo(on_wait=[w], on_update=[]),
                            )
                        )
                    inst.sync_info = mybir.SyncInfo(
                        on_wait=[waits[-1]], on_update=list(si.on_update)
                    )
                if si is not None and len(si.on_update) > 1:
                    raise RuntimeError(
                        f"{inst.name}: {len(si.on_update)} sem updates unsupported"
                    )
                out.append(inst)
            bb.instructions = out


def _build():
    from contextlib import ExitStack

    nc = bass.Bass("TRN2", target_bir_lowering=False, debug=False, num_devices=1)

    xT_d = nc.dram_tensor("xT", [128, DC * S], bf16, kind="ExternalInput").ap()
    w_d = nc.dram_tensor("w", [128, DC * 3 * EL], bf16, kind="ExternalInput").ap()
    wp_d = nc.dram_tensor("wp", [128, WC * D], bf16, kind="ExternalInput").ap()
    bqk_d = nc.dram_tensor("bqk", [128, 2 * WC], f32, kind="ExternalInput").ap()
    tri_d = nc.dram_tensor("tri", [128, 256], bf16, kind="ExternalInput").ap()
    yT_d = nc.dram_tensor("yT", [D, S], f32, kind="ExternalOutput").ap()

    xT_v = xT_d.rearrange("p (c s) -> p c s", s=S)
    w_v = w_d.rearrange("p (c e) -> p c e", e=3 * EL)
    wp_v = wp_d.rearrange("p (c o) -> p c o", o=D)

    with tile.TileContext(nc) as tc, ExitStack() as ctx:
        const = ctx.enter_context(tc.tile_pool(name="const", bufs=1))
        xp = ctx.enter_context(tc.tile_pool(name="xp", bufs=2))
        qp = ctx.enter_context(tc.tile_pool(name="qp", bufs=2))
        kp = ctx.enter_context(tc.tile_pool(name="kp", bufs=1))
        vp = ctx.enter_context(tc.tile_pool(name="vp", bufs=1))
        ap_ = ctx.enter_context(tc.tile_pool(name="ap", bufs=4))
        rp = ctx.enter_context(tc.tile_pool(name="rp", bufs=2))
        bp = ctx.enter_context(tc.tile_pool(name="bp", bufs=2))
        op = ctx.enter_context(tc.tile_pool(name="op", bufs=2))
        yp = ctx.enter_context(tc.tile_pool(name="yp", bufs=2))
        dp = ctx.enter_context(tc.tile_pool(name="dp", bufs=2, space="DRAM"))
        ps2 = ctx.enter_context(tc.tile_pool(name="ps2", bufs=3, space="PSUM"))
        ps1 = ctx.enter_context(tc.tile_pool(name="ps1", bufs=2, space="PSUM"))

        w_sb = const.tile([128, DC, 3 * EL], bf16)
        nc.sync.dma_start(w_sb[:], w_v[:])
        wp_sb = const.tile([128, WC, D], bf16)
        nc.sync.dma_start(wp_sb[:], wp_v[:])
        bqk_sb = const.tile([128, 2 * WC], f32)
        nc.sync.dma_start(bqk_sb[:], bqk_d[:])
        tri_sb = const.tile([128, 2, 128], bf16)
        nc.sync.dma_start(tri_sb[:], tri_d.rearrange("p (i n) -> p i n", i=2))

        kTs, vts, qTs, outTps = [], [], [], []

        def emit_qkv(j):
            """Generator of emission thunks: QKV for macro j."""
            xT_j = xp.tile([128, DC, MACRO], bf16)
            nc.sync.dma_start(xT_j[:], xT_v[:, :, j * MACRO:(j + 1) * MACRO])
            qT = qp.tile([128, WC, MACRO], bf16)
            kT = kp.tile([128, WC, MACRO], bf16, tag=f"k{j}")
            qTs.append(qT)
            kTs.append(kT)

            def qk_tile(m):
                ps = ps1.tile([128, 512], f32, tag="ps1")
                for dc in range(DC):
                    nc.tensor.matmul(
                        ps[:],
                        w_sb[:, dc, m * 128:(m + 1) * 128],
                        xT_j[:, dc, :],
                        start=(dc == 0),
                        stop=(dc == DC - 1),
                    )
                dst = qT[:, m, :] if m < WC else kT[:, m - WC, :]
                nc.vector.tensor_scalar_add(dst, ps[:], bqk_sb[:, m:m + 1])

            for m in range(2 * WC):
                yield lambda m=m: qk_tile(m)

            vt = vp.tile([128, CPM, HL * (HD + 1)], bf16, tag=f"v{j}")
            vts.append(vt)

            def v_tile(a):
                vps = ps2.tile([128, 1024], f32, tag="ps2")
                for dc in range(DC):
                    nc.tensor.matmul(
                        vps[:, 0:512],
                        xT_j[:, dc, a * 128:(a + 1) * 128],
                        w_sb[:, dc, 2 * EL:2 * EL + 512],
                        start=(dc == 0),
                        stop=(dc == DC - 1),
                    )
                for dc in range(DC):
                    nc.tensor.matmul(
                        vps[:, 512:640],
                        xT_j[:, dc, a * 128:(a + 1) * 128],
                        w_sb[:, dc, 2 * EL + 512:3 * EL],
                        start=(dc == 0),
                        stop=(dc == DC - 1),
                    )
                vt_v = vt[:, a, :].rearrange("p (h c) -> p h c", c=HD + 1)
                vps_v = vps[:, 0:EL].rearrange("p (h c) -> p h c", c=HD)
                nc.vector.tensor_copy(vt_v[:, :, 0:HD], vps_v[:])

            for a in range(CPM):
                yield lambda a=a: v_tile(a)

            def ones_fill():
                ones_v = vt[:].rearrange("p a (h c) -> p a h c", c=HD + 1)
                nc.vector.memset(ones_v[:, :, :, HD:HD + 1], 1.0)

            yield ones_fill

        def emit_attn(j):
            """Generator of emission thunks: attention for macro j."""
            qT = qTs[j]
            outTp = op.tile([128, WC, MACRO], bf16)
            outTps.append(outTp)
            nch = CPM * (j + 1)

            for t in range(PAIRS):
                outps = ps2.tile([128, 1024], f32, tag="ps2")
                outps_v = outps[:].rearrange("p (i n) -> p i n", i=2)

                def chunk(t, c, outps, outps_v):
                    q0 = max(0, 128 * c - j * MACRO)
                    n = MACRO - q0
                    km, vm, cc = kTs[c // CPM], vts[c // CPM], c % CPM
                    sps = ps2.tile([128, 1024], f32, tag="ps2")
                    sps_v = sps[:].rearrange("p (i n) -> p i n", i=2)
                    nc.tensor.matmul(
                        sps[:, 0:n],
                        km[0:64, t, cc * 128:(cc + 1) * 128],
                        qT[0:64, t, q0:MACRO],
                        start=True, stop=True, tile_position=(0, 0),
                    )
                    nc.tensor.matmul(
                        sps[:, 512:512 + n],
                        km[64:128, t, cc * 128:(cc + 1) * 128],
                        qT[64:128, t, q0:MACRO],
                        start=True, stop=True, tile_position=(64, 0),
                    )
                    at = ap_.tile([128, 2, 512], bf16, tag="at")
                    nc.scalar.activation(
                        at[:, :, 0:n], sps_v[:, :, 0:n], EXP, scale=0.125
                    )
                    if c >= CPM * j:  # diagonal chunk: mask first 128 cols
                        nc.vector.tensor_mul(
                            at[:, :, 0:128], at[:, :, 0:128], tri_sb[:]
                        )
                    nc.tensor.matmul(
                        outps[0:65, q0:MACRO],
                        vm[:, cc, 2 * t * (HD + 1):(2 * t + 1) * (HD + 1)],
                        at[:, 0, 0:n],
                        start=(c == 0), stop=(c == nch - 1),
                    )
                    nc.tensor.matmul(
                        outps_v[0:65, 1, q0:MACRO],
                        vm[:, cc, (2 * t + 1) * (HD + 1):(2 * t + 2) * (HD + 1)],
                        at[:, 1, 0:n],
                        start=(c == 0), stop=(c == nch - 1),
                    )

                for c in range(nch):
                    yield lambda t=t, c=c, o=outps, ov=outps_v: chunk(t, c, o, ov)

                def normalize(t, outps, outps_v):
                    rc = rp.tile([1, 2, 512], f32, tag="rc")
                    nc.vector.reciprocal(rc[:], outps_v[64:65, :, :])
                    bounce = dp.tile([1, 1024], f32, tag="bounce")
                    nc.gpsimd.dma_start(bounce[:], rc[:].rearrange("p i n -> p (i n)"))
                    bc = bp.tile([64, 2, 512], f32, tag="bc")
                    nc.gpsimd.dma_start(
                        bc[:],
                        bounce.rearrange("p (i n) -> p i n", i=2).to_broadcast((64, 2, 512)),
                    )
                    nc.vector.tensor_mul(
                        outTp[0:64, t, :], outps_v[0:64, 0, :], bc[:, 0, :]
                    )
                    nc.vector.tensor_mul(
                        outTp[64:128, t, :], outps_v[0:64, 1, :], bc[:, 1, :]
                    )

                yield lambda t=t, o=outps, ov=outps_v: normalize(t, o, ov)

        def emit_proj(j):
            """Generator of emission thunks: projection for macro j."""
            outTp = outTps[j]

            def n_tile(n):
                yps = ps1.tile([128, 512], f32, tag="ps1")
                for dc in range(WC):
                    nc.tensor.matmul(
                        yps[:],
                        wp_sb[:, dc, n * 128:(n + 1) * 128],
                        outTp[:, dc, :],
                        start=(dc == 0),
                        stop=(dc == WC - 1),
                    )
                ysb = yp.tile([128, MACRO], f32, tag="ysb")
                nc.vector.tensor_copy(ysb[:], yps[:])
                nc.sync.dma_start(
                    yT_d[n * 128:(n + 1) * 128, j * MACRO:(j + 1) * MACRO], ysb[:]
                )

            for n in range(DC):
                yield lambda n=n: n_tile(n)

        def interleave(primary, *others):
            """Emit primary thunks, spreading each `others` list evenly."""
            prim = list(primary)
            rest = [th for o in others for th in o]
            np_, nr = len(prim), len(rest)
            ri = 0
            for i, th in enumerate(prim):
                th()
                want = (i + 1) * nr // np_ if np_ else nr
                while ri < want:
                    rest[ri]()
                    ri += 1
            while ri < nr:
                rest[ri]()
                ri += 1

        # prologue: QKV(0)
        for th in emit_qkv(0):
            th()
        # steady: attn(j) paced against QKV(j+1) and proj(j-1)
        for j in range(NM):
            others = []
            if j + 1 < NM:
                others.append(list(emit_qkv(j + 1)))
            if j - 1 >= 0:
                others.append(list(emit_proj(j - 1)))
            interleave(emit_attn(j), *others)
        for th in emit_proj(NM - 1):
            th()

    _fix_sync_caps(nc)
    return nc


def _to_bf16(a):
    return np.ascontiguousarray(a).astype(ml_dtypes.bfloat16)


def _chunk_rows(a, nchunks):
    """[nchunks*128, F] -> [128, nchunks*F] with [p, c*F+f] = a[c*128+p, f]."""
    f = a.shape[1]
    return np.ascontiguousarray(
        a.reshape(nchunks, 128, f).transpose(1, 0, 2).reshape(128, nchunks * f)
    )


def kernel(hidden_states, w_attn, b_attn, w_proj, b_proj):
    x = np.asarray(hidden_states, dtype=np.float32)
    wa = np.asarray(w_attn, dtype=np.float32)
    ba = np.asarray(b_attn, dtype=np.float32)
    wp = np.asarray(w_proj, dtype=np.float32)
    bp_ = np.asarray(b_proj, dtype=np.float32)

    if "nc" not in _CACHE:
        _CACHE["nc"] = _build()
    nc = _CACHE["nc"]

    tri = np.triu(np.ones((128, 128), np.float32))
    tri2 = _to_bf16(np.concatenate([tri, tri], axis=1))

    in_maps = []
    for core in range(N_CORES):
        b, g = core // 2, core % 2
        e0 = EL * g
        w_loc = np.concatenate(
            [wa[:, e0:e0 + EL], wa[:, D + e0:D + e0 + EL],
             wa[:, 2 * D + e0:2 * D + e0 + EL]], axis=1
        )  # [1280, 1920]
        bqk = np.concatenate([ba[e0:e0 + EL], ba[D + e0:D + e0 + EL]])
        in_maps.append({
            "xT": _to_bf16(_chunk_rows(np.ascontiguousarray(x[b].T), DC)),
            "w": _to_bf16(_chunk_rows(w_loc, DC)),
            "wp": _to_bf16(_chunk_rows(wp[e0:e0 + EL, :], WC)),
            "bqk": np.ascontiguousarray(bqk.reshape(2 * WC, 128).T),
            "tri": tri2,
        })

    _CACHE["last_in_maps"] = in_maps
    res = bass_utils.run_bass_kernel_spmd(
        nc, in_maps, core_ids=list(range(N_CORES))
    )

    bias = bp_ + ba[2 * D:] @ wp  # v-bias folded through the projection
    y = np.empty((B, S, D), np.float32)
    for b in range(B):
        yT = res.results[2 * b]["yT"] + res.results[2 * b + 1]["yT"]
        y[b] = yT.T + bias
    return y

